# revision 25
# baseline (speedup 1.0000x reference)
"""Multi-head causal attention (B=4, T=2048, DM=1024, H=16, dk=dv=64) on 8
Trainium2 NeuronCores.

Sharding: core c handles batch b = c//2 and head-group g = c%2 (8 heads).
Data-parallel over batch x tensor-parallel over heads; no cross-core comm.

Per-core bass/Tile kernel (all matmuls bf16, PSUM accumulation fp32):
  - host pre-lays-out x^T (d on partitions, chunk-major), Wq||Wk stacked
    per head, Wv packed across heads, and the causal mask tiles, in bf16.
  - projections: qT/kT = (Wq||Wk)^T-stationary matmuls vs x^T;
    v in natural [t, dv] layout via x^T-stationary matmuls vs packed Wv.
  - attention, flash-style over 512-wide t-chunks and 128-wide s-tiles:
      S^T[s,t] = kT_slice.T @ qT_chunk          (PE, K=64, row-tiled pairs)
      P = exp(S * dk^-0.5)                       (ScalarE, scale folded in)
      diagonal tiles: P *= causal 0/1 mask       (VectorE)
      O_aug^T[65, t] += [v | 1]^T-stationary @ P (PE, K=128, fp32 accum)
    row 64 of O_aug^T collects the softmax denominators.
  - O_aug^T chunks are copied to SBUF (bf16) and DMAed out unnormalized;
    the host does the final divide + transpose (O(T*DV) work).

Schedule: the kernel is PE-bound (~1000 matmul slots x ~214ns) and the 7MB
input prefetch takes ~20us of DMA bandwidth, so attention runs
CHUNK-COLUMN-major: every head's chunk-c attention needs only x chunks
<= c, which gives the x prefetch a full c=0 sweep of runway.  All
projection matmuls sit in a fine-grained work queue and are pumped ~2
instructions after every attention pair so they fill the exp-wait gaps;
flush barriers guarantee each attention chunk's inputs are emitted (=
higher scheduler priority) before the chunk that needs them.
"""
import numpy as np
import ml_dtypes

_BF16 = ml_dtypes.bfloat16

B, T, DM = 4, 2048, 1024
H, DK, DV = 16, 64, 64
N_CORES = 8
HPC = 8          # heads per core
NDC = DM // 128  # 8 d-chunks
NTT = T // 128   # 16 t/s tiles of 128
NTC = T // 512   # 4 t-chunks of 512

_cached = None   # (nc, run_bass_kernel_spmd)

# Set by a driver (e.g. test.py) to collect an NTFF profile; the exec time
# lands in LAST_EXEC_NS.
TRACE = False
LAST_EXEC_NS = None


def _build_program():
    global _cached
    if _cached is not None:
        return _cached
    import concourse.bacc as bacc
    import concourse.mybir as mybir
    from concourse import tile

    bf16 = mybir.dt.bfloat16
    f32 = mybir.dt.float32
    Exp = mybir.ActivationFunctionType.Exp

    nc = bacc.Bacc()
    # xt is chunk-major so each 512-wide t-chunk is ONE contiguous DMA
    # descriptor: descriptor issue on the sync queue costs ~700ns each, so
    # the input prefetch must be few large transfers, not 49 small ones.
    xt = nc.declare_dram_parameter("xt", [128, NTC, NDC, 512], bf16, isOutput=False)
    wqk = nc.declare_dram_parameter("wqk", [128, HPC, NDC, 128], bf16, isOutput=False)
    wv = nc.declare_dram_parameter("wv", [128, NDC, 512], bf16, isOutput=False)
    msk = nc.declare_dram_parameter("msk", [128, 1280], bf16, isOutput=False)
    ot = nc.declare_dram_parameter("ot", [HPC, DV + 1, T], bf16, isOutput=True)

    with tile.TileContext(nc) as tc:
        with (
            tc.tile_pool(name="consts", bufs=1) as consts,
            tc.tile_pool(name="vpool", bufs=1) as vpool,
            tc.tile_pool(name="qk", bufs=8) as qkpool,
            tc.tile_pool(name="pt", bufs=4) as ptpool,
            tc.tile_pool(name="osb", bufs=8) as opool,
            tc.tile_pool(name="proj_ps", bufs=2, space="PSUM") as proj_ps,
            tc.tile_pool(name="s_ps", bufs=2, space="PSUM") as s_ps,
            tc.tile_pool(name="o_ps", bufs=2, space="PSUM") as o_ps,
        ):
            # Seven coarse input transfers, ordered by first use: x chunk 0
            # + wv feed the V-proj preamble, wqk (all heads at once) feeds
            # every QK-proj, the remaining x chunks stream in behind.
            wv_sb = consts.tile([128, NDC, 512], bf16)
            msk_sb = consts.tile([128, 1280], bf16)
            xt_sb = consts.tile([128, NTC, NDC, 512], bf16)
            wqk_sb = consts.tile([128, HPC, NDC, 128], bf16)
            nc.sync.dma_start(xt_sb[:, 0], xt[:, 0])
            nc.sync.dma_start(wv_sb[:], wv[:])
            nc.sync.dma_start(wqk_sb[:], wqk[:])
            nc.sync.dma_start(msk_sb[:], msk[:])
            for tch in range(1, NTC):
                nc.sync.dma_start(xt_sb[:, tch], xt[:, tch])

            # HAM warmup: dependency-free matmuls on memset tiles run during
            # the initial DMA wait, so the PE clock gate is already at 8/8
            # when the real matmuls start (~3.4us of sustained activity).
            wu_w = consts.tile([128, 128], bf16)
            wu_x = consts.tile([128, 512], bf16)
            nc.gpsimd.memset(wu_w[:], 0.0)
            nc.gpsimd.memset(wu_x[:], 0.0)
            for i in range(9):
                ps = proj_ps.tile([128, 512], f32, name="ps_wu", tag="ps_qk")
                nc.tensor.matmul(ps[:], lhsT=wu_w[:], rhs=wu_x[:],
                                 start=True, stop=True)
            # also pull the ~2.7us exp ACT_TABLE_LOAD into the DMA wait
            wu_e = consts.tile([128, 512], bf16)
            nc.scalar.activation(wu_e[:], wu_x[:], Exp)

            # V projection, emitted per t-tile so it can interleave with
            # attention: v_sb[s, j, h, 0:64] = v values, v_sb[s, j, h, 64]
            # = 1.0 (softmax-denominator column).
            v_sb = vpool.tile([128, NTT, HPC, DV + 1], bf16)
            nc.gpsimd.memset(v_sb[:, :, :, DV], 1.0)

            # Projection work is managed as a fine-grained queue of closures
            # (one matmul or copy each) so the emitter can interleave a few
            # projection instructions after every attention pair — matching
            # the ~2-slot PE deficit of each exp-paced pair — instead of
            # dropping 8-matmul blobs between chunks.
            proj_work = []   # items: callable | ('marker', key)
            _done_markers = set()

            def flush_until(key):
                while key not in _done_markers:
                    it = proj_work.pop(0)
                    if isinstance(it, tuple):
                        _done_markers.add(it[1])
                    else:
                        it()

            def pump(n):
                while n > 0 and proj_work:
                    it = proj_work.pop(0)
                    if isinstance(it, tuple):
                        _done_markers.add(it[1])
                    else:
                        it()
                        n -= 1

            def enq_v_proj(tt):
                state = {}

                def mm(dc):
                    def go():
                        if dc == 0:
                            state['ps'] = proj_ps.tile(
                                [128, 512], f32, name="ps_v", tag="ps_qk")
                        nc.tensor.matmul(
                            state['ps'][:],
                            lhsT=xt_sb[:, tt // 4, dc,
                                       128 * (tt % 4):128 * (tt % 4 + 1)],
                            rhs=wv_sb[:, dc, :],
                            start=(dc == 0),
                            stop=(dc == NDC - 1),
                        )
                    return go

                def cp():
                    nc.vector.tensor_copy(
                        v_sb[:, tt, :, 0:DV],
                        state['ps'][:].rearrange("p (h e) -> p h e", h=HPC),
                    )

                proj_work.extend([mm(dc) for dc in range(NDC)])
                proj_work.append(cp)
                proj_work.append(('marker', ('v', tt)))

            def qk_alloc(h):
                # qk1 = [q | k] on partitions [0:64 | 64:128];
                # qk2 = [k | q] (swapped halves).  Row-packed S matmuls need
                # weights and fmap at the SAME base partition, so even s-tiles
                # use (k,q) from partitions 0:64 and odd s-tiles use (k,q)
                # from partitions 64:128.
                qk1 = qkpool.tile([128, T], bf16, name=f"qk1_{h}", tag="qk1")
                qk2 = qkpool.tile([128, T], bf16, name=f"qk2_{h}", tag="qk2")
                return qk1, qk2

            def enq_qk_proj_chunk(h, qk, tch):
                # QK projection for head h, one 512-wide t-chunk: psum rows
                # 0:64 hold the q^T chunk, rows 64:128 the k^T chunk.  The
                # 1:1 LDWEIGHTS:MATMUL ratio is free — weight loads for
                # N=512 streams hide entirely behind the previous matmul.
                qk1, qk2 = qk
                state = {}
                sl = slice(512 * tch, 512 * (tch + 1))

                def mm(dc):
                    def go():
                        if dc == 0:
                            state['ps'] = proj_ps.tile(
                                [128, 512], f32, name="ps_qk", tag="ps_qk")
                        nc.tensor.matmul(
                            state['ps'][:],
                            lhsT=wqk_sb[:, h, dc, :],
                            rhs=xt_sb[:, tch, dc, :],
                            start=(dc == 0),
                            stop=(dc == NDC - 1),
                        )
                    return go

                def cp():
                    nc.vector.tensor_copy(qk1[:, sl], state['ps'][:])
                    # swapped halves, cheap SBUF->SBUF bf16 copies
                    nc.vector.tensor_copy(qk2[0:64, sl], qk1[64:128, sl])
                    nc.vector.tensor_copy(qk2[64:128, sl], qk1[0:64, sl])

                proj_work.extend([mm(dc) for dc in range(NDC)])
                proj_work.append(cp)
                proj_work.append(('marker', ('qk', h, tch)))

            def attn_chunk(h, qk1, qk2, c, o_sb):
                # Attention for head h, one 512-wide t-chunk, causal.
                po = o_ps.tile([DV + 1, 512], f32, name="po", tag="po")
                jmax = 4 * c + 3        # last s-tile index for this chunk
                csl = slice(512 * c, 512 * (c + 1))
                for pair in range(2 * (c + 1)):
                    pS = s_ps.tile([128, 1024], f32, name="pS", tag="pS")
                    pt = ptpool.tile([128, 1024], bf16, name="pt", tag="pt")
                    j0 = 2 * pair
                    # Diagonal s-tiles (relative index r = j - 4c in 0..3) are
                    # fully masked below t-offset 128*r, so S / exp / PV only
                    # cover t in [128*r, 512).  The u=1 tile's output is
                    # COMPACTED to start at psum col 512 so the pair's live
                    # region [f0A : 1024-f0B] stays contiguous and a single
                    # exp op covers it.
                    rA = j0 - 4 * c
                    rB = rA + 1
                    f0A = max(0, 128 * rA)
                    f0B = max(0, 128 * rB)
                    nc.tensor.matmul(
                        pS[:, f0A:512],
                        lhsT=qk2[0:64, 128 * j0:128 * (j0 + 1)],
                        rhs=qk1[0:64, 512 * c + f0A:512 * (c + 1)],
                        start=True,
                        stop=True,
                        tile_position=(0, 0),
                    )
                    nc.tensor.matmul(
                        pS[:, 512:1024 - f0B],
                        lhsT=qk1[64:128, 128 * (j0 + 1):128 * (j0 + 2)],
                        rhs=qk2[64:128, 512 * c + f0B:512 * (c + 1)],
                        start=True,
                        stop=True,
                        tile_position=(64, 0),
                    )
                    nc.scalar.activation(
                        pt[:, f0A:1024 - f0B], pS[:, f0A:1024 - f0B],
                        Exp, scale=DK ** -0.5,
                    )
                    if rA >= 0:
                        # diagonal pair: one multiply with the pre-packed
                        # causal mask (d0 pair at mask cols 0:896, d1 pair at
                        # 896:1280, both laid out to match the compacted pt).
                        m0 = 0 if rA == 0 else 896
                        mw = 896 if rA == 0 else 384
                        nc.vector.tensor_mul(
                            pt[:, f0A:1024 - f0B], pt[:, f0A:1024 - f0B],
                            msk_sb[:, m0:m0 + mw],
                        )
                    for u, j, f0 in ((0, j0, f0A), (1, j0 + 1, f0B)):
                        nc.tensor.matmul(
                            po[:, f0:512],
                            lhsT=v_sb[:, j, h, :],
                            rhs=pt[:, 512 * u + (f0 if u == 0 else 0):
                                   512 * (u + 1) - (0 if u == 0 else f0)],
                            start=(j == 0),
                            stop=(j == jmax),
                        )
                    # fill this pair's exp-wait gap with ~2 projection slots
                    pump(2)
                nc.vector.tensor_copy(o_sb[:, csl], po[:])
                # per-chunk output DMA keeps the sync-queue issue cost
                # (~700ns each) spread across the kernel instead of piled
                # at the end.
                nc.sync.dma_start(ot[h, :, csl], o_sb[:, csl])

            # Emission order = scheduler priority (see module docstring).
            qks = {h: qk_alloc(h) for h in range(HPC)}
            osbs = {
                h: opool.tile([DV + 1, T], bf16, name=f"o_sb{h}", tag="o_sb")
                for h in range(HPC)
            }
            for tt in range(4):
                enq_v_proj(tt)
            enq_qk_proj_chunk(0, qks[0], 0)
            flush_until(('qk', 0, 0))
            # enqueue the rest in consumption order: remaining heads'
            # chunk-0 projections, then per chunk column c: the v tiles and
            # qk chunks that column consumes.
            for h in range(1, HPC):
                enq_qk_proj_chunk(h, qks[h], 0)
            for c in range(1, NTC):
                for tt in range(4 * c, 4 * (c + 1)):
                    enq_v_proj(tt)
                for h in range(HPC):
                    enq_qk_proj_chunk(h, qks[h], c)
            for c in range(NTC):
                for h in range(HPC):
                    if c > 0 and h == 0:
                        flush_until(('v', 4 * c + 3))
                    flush_until(('qk', h, c))
                    attn_chunk(h, *qks[h], c, osbs[h])
            while proj_work:
                pump(1)

    nc.finalize()
    from concourse.bass_utils import run_bass_kernel_spmd
    _cached = (nc, run_bass_kernel_spmd)
    return _cached


def _prep_core_inputs(x, Wq, Wk, Wv, core):
    b, g = core // 2, core % 2
    xb = x[b].astype(_BF16)                                  # [T, DM]
    xt = np.ascontiguousarray(
        xb.T.reshape(NDC, 128, NTC, 512).transpose(1, 2, 0, 3)  # [p, c, dc, t]
    )
    wq = Wq[HPC * g:HPC * (g + 1)].astype(_BF16)             # [8, DM, 64]
    wk = Wk[HPC * g:HPC * (g + 1)].astype(_BF16)
    wv = Wv[HPC * g:HPC * (g + 1)].astype(_BF16)
    wqk = np.concatenate([wq, wk], axis=2)                   # [h, DM, 128]
    wqk = np.ascontiguousarray(
        wqk.reshape(HPC, NDC, 128, 128).transpose(2, 0, 1, 3)  # [p, h, dc, f]
    )
    wvp = np.ascontiguousarray(
        wv.reshape(HPC, NDC, 128, DV).transpose(2, 1, 0, 3).reshape(128, NDC, 512)
    )
    return {"xt": xt, "wqk": wqk, "wv": wvp, "msk": _mask()}


_mask_cache = None


def _mask():
    # Packed causal masks matching the compacted diagonal-pair layout:
    # cols 0:896  = d0 pair: [r0 tile t 0:512 | r1 tile t 128:512]
    # cols 896:1280 = d1 pair: [r2 tile t 256:512 | r3 tile t 384:512]
    global _mask_cache
    if _mask_cache is None:
        p = np.arange(128)[:, None]
        m = np.zeros((128, 1280), np.float32)
        c = np.arange(512)[None, :]
        m[:, 0:512] = p <= c
        c = np.arange(384)[None, :]
        m[:, 512:896] = p <= c
        c = np.arange(256)[None, :]
        m[:, 896:1152] = p <= c
        c = np.arange(128)[None, :]
        m[:, 1152:1280] = p <= c
        _mask_cache = m.astype(_BF16)
    return _mask_cache


def kernel(x, Wq, Wk, Wv):
    global LAST_EXEC_NS
    nc, run_spmd = _build_program()
    in_maps = [_prep_core_inputs(x, Wq, Wk, Wv, c) for c in range(N_CORES)]
    res = run_spmd(nc, in_maps, list(range(N_CORES)), trace=TRACE)
    global _LAST_RES
    _LAST_RES = res
    LAST_EXEC_NS = res.exec_time_ns

    out = np.empty((B, T, H * DV), np.float32)
    for c in range(N_CORES):
        b, g = c // 2, c % 2
        otc = res.results[c]["ot"].astype(np.float32)  # [8, 65, T]
        o = otc[:, :DV, :] / otc[:, DV:DV + 1, :]      # [h, dv, t]
        out[b, :, 512 * g:512 * (g + 1)] = (
            o.transpose(2, 0, 1).reshape(T, HPC * DV)
        )
    return out


# revision 26
# speedup vs baseline: 1.0076x; 1.0076x over previous
"""Multi-head causal attention (B=4, T=2048, DM=1024, H=16, dk=dv=64) on 8
Trainium2 NeuronCores.

Sharding: core c handles batch b = c//2 and head-group g = c%2 (8 heads).
Data-parallel over batch x tensor-parallel over heads; no cross-core comm.

Per-core bass/Tile kernel (all matmuls bf16, PSUM accumulation fp32):
  - host pre-lays-out x^T (d on partitions, chunk-major), Wq||Wk stacked
    per head, Wv packed across heads, and the causal mask tiles, in bf16.
  - projections: qT/kT = (Wq||Wk)^T-stationary matmuls vs x^T;
    v in natural [t, dv] layout via x^T-stationary matmuls vs packed Wv.
  - attention, flash-style over 512-wide t-chunks and 128-wide s-tiles:
      S^T[s,t] = kT_slice.T @ qT_chunk          (PE, K=64, row-tiled pairs)
      P = exp(S * dk^-0.5)                       (ScalarE, scale folded in)
      diagonal tiles: P *= causal 0/1 mask       (VectorE)
      O_aug^T[65, t] += [v | 1]^T-stationary @ P (PE, K=128, fp32 accum)
    row 64 of O_aug^T collects the softmax denominators.
  - O_aug^T chunks are copied to SBUF (bf16) and DMAed out unnormalized;
    the host does the final divide + transpose (O(T*DV) work).

Schedule: the kernel is PE-bound (~1000 matmul slots x ~214ns) and the 7MB
input prefetch takes ~20us of DMA bandwidth, so attention runs
CHUNK-COLUMN-major: every head's chunk-c attention needs only x chunks
<= c, which gives the x prefetch a full c=0 sweep of runway.  All
projection matmuls sit in a fine-grained work queue and are pumped ~2
instructions after every attention pair so they fill the exp-wait gaps;
flush barriers guarantee each attention chunk's inputs are emitted (=
higher scheduler priority) before the chunk that needs them.
"""
import numpy as np
import ml_dtypes

_BF16 = ml_dtypes.bfloat16

B, T, DM = 4, 2048, 1024
H, DK, DV = 16, 64, 64
N_CORES = 8
HPC = 8          # heads per core
NDC = DM // 128  # 8 d-chunks
NTT = T // 128   # 16 t/s tiles of 128
NTC = T // 512   # 4 t-chunks of 512

_cached = None   # (nc, run_bass_kernel_spmd)

# Set by a driver (e.g. test.py) to collect an NTFF profile; the exec time
# lands in LAST_EXEC_NS.
TRACE = False
LAST_EXEC_NS = None


def _build_program():
    global _cached
    if _cached is not None:
        return _cached
    import concourse.bacc as bacc
    import concourse.mybir as mybir
    from concourse import tile

    bf16 = mybir.dt.bfloat16
    f32 = mybir.dt.float32
    Exp = mybir.ActivationFunctionType.Exp

    nc = bacc.Bacc()
    # xt is chunk-major so each 512-wide t-chunk is ONE contiguous DMA
    # descriptor: descriptor issue on the sync queue costs ~700ns each, so
    # the input prefetch must be few large transfers, not 49 small ones.
    xt = nc.declare_dram_parameter("xt", [128, NTC, NDC, 512], bf16, isOutput=False)
    wqk = nc.declare_dram_parameter("wqk", [128, HPC, NDC, 128], bf16, isOutput=False)
    wv = nc.declare_dram_parameter("wv", [128, NDC, 512], bf16, isOutput=False)
    msk = nc.declare_dram_parameter("msk", [128, 1280], bf16, isOutput=False)
    ot = nc.declare_dram_parameter("ot", [HPC, DV + 1, T], bf16, isOutput=True)

    with tile.TileContext(nc) as tc:
        with (
            tc.tile_pool(name="consts", bufs=1) as consts,
            tc.tile_pool(name="vpool", bufs=1) as vpool,
            tc.tile_pool(name="qk", bufs=8) as qkpool,
            tc.tile_pool(name="pt", bufs=4) as ptpool,
            tc.tile_pool(name="osb", bufs=8) as opool,
            tc.tile_pool(name="proj_ps", bufs=2, space="PSUM") as proj_ps,
            tc.tile_pool(name="s_ps", bufs=2, space="PSUM") as s_ps,
            tc.tile_pool(name="o_ps", bufs=2, space="PSUM") as o_ps,
        ):
            # Seven coarse input transfers, ordered by first use: x chunk 0
            # + wv feed the V-proj preamble, wqk (all heads at once) feeds
            # every QK-proj, the remaining x chunks stream in behind.
            wv_sb = consts.tile([128, NDC, 512], bf16)
            msk_sb = consts.tile([128, 1280], bf16)
            xt_sb = consts.tile([128, NTC, NDC, 512], bf16)
            wqk_sb = consts.tile([128, HPC, NDC, 128], bf16)
            nc.sync.dma_start(xt_sb[:, 0], xt[:, 0])
            nc.sync.dma_start(wv_sb[:], wv[:])
            nc.sync.dma_start(wqk_sb[:], wqk[:])
            nc.sync.dma_start(msk_sb[:], msk[:])
            for tch in range(1, NTC):
                nc.sync.dma_start(xt_sb[:, tch], xt[:, tch])

            # HAM warmup: dependency-free matmuls on memset tiles run during
            # the initial DMA wait, so the PE clock gate is already at 8/8
            # when the real matmuls start (~3.4us of sustained activity).
            wu_w = consts.tile([128, 128], bf16)
            wu_x = consts.tile([128, 512], bf16)
            nc.gpsimd.memset(wu_w[:], 0.0)
            nc.gpsimd.memset(wu_x[:], 0.0)
            for i in range(12):
                ps = proj_ps.tile([128, 512], f32, name="ps_wu", tag="ps_qk")
                nc.tensor.matmul(ps[:], lhsT=wu_w[:], rhs=wu_x[:],
                                 start=True, stop=True)
            # also pull the ~2.7us exp ACT_TABLE_LOAD into the DMA wait
            wu_e = consts.tile([128, 512], bf16)
            nc.scalar.activation(wu_e[:], wu_x[:], Exp)

            # V projection, emitted per t-tile so it can interleave with
            # attention: v_sb[s, j, h, 0:64] = v values, v_sb[s, j, h, 64]
            # = 1.0 (softmax-denominator column).
            v_sb = vpool.tile([128, NTT, HPC, DV + 1], bf16)
            nc.gpsimd.memset(v_sb[:, :, :, DV], 1.0)

            # Projection work is managed as a fine-grained queue of closures
            # (one matmul or copy each) so the emitter can interleave a few
            # projection instructions after every attention pair — matching
            # the ~2-slot PE deficit of each exp-paced pair — instead of
            # dropping 8-matmul blobs between chunks.
            proj_work = []   # items: callable | ('marker', key)
            _done_markers = set()

            def flush_until(key):
                while key not in _done_markers:
                    it = proj_work.pop(0)
                    if isinstance(it, tuple):
                        _done_markers.add(it[1])
                    else:
                        it()

            def pump(n):
                while n > 0 and proj_work:
                    it = proj_work.pop(0)
                    if isinstance(it, tuple):
                        _done_markers.add(it[1])
                    else:
                        it()
                        n -= 1

            def enq_v_proj(tt):
                state = {}

                def mm(dc):
                    def go():
                        if dc == 0:
                            state['ps'] = proj_ps.tile(
                                [128, 512], f32, name="ps_v", tag="ps_qk")
                        nc.tensor.matmul(
                            state['ps'][:],
                            lhsT=xt_sb[:, tt // 4, dc,
                                       128 * (tt % 4):128 * (tt % 4 + 1)],
                            rhs=wv_sb[:, dc, :],
                            start=(dc == 0),
                            stop=(dc == NDC - 1),
                        )
                    return go

                def cp():
                    nc.vector.tensor_copy(
                        v_sb[:, tt, :, 0:DV],
                        state['ps'][:].rearrange("p (h e) -> p h e", h=HPC),
                    )

                proj_work.extend([mm(dc) for dc in range(NDC)])
                proj_work.append(cp)
                proj_work.append(('marker', ('v', tt)))

            def qk_alloc(h):
                # qk1 = [q | k] on partitions [0:64 | 64:128];
                # qk2 = [k | q] (swapped halves).  Row-packed S matmuls need
                # weights and fmap at the SAME base partition, so even s-tiles
                # use (k,q) from partitions 0:64 and odd s-tiles use (k,q)
                # from partitions 64:128.
                qk1 = qkpool.tile([128, T], bf16, name=f"qk1_{h}", tag="qk1")
                qk2 = qkpool.tile([128, T], bf16, name=f"qk2_{h}", tag="qk2")
                return qk1, qk2

            def enq_qk_proj_chunk(h, qk, tch):
                # QK projection for head h, one 512-wide t-chunk: psum rows
                # 0:64 hold the q^T chunk, rows 64:128 the k^T chunk.  The
                # 1:1 LDWEIGHTS:MATMUL ratio is free — weight loads for
                # N=512 streams hide entirely behind the previous matmul.
                qk1, qk2 = qk
                state = {}
                sl = slice(512 * tch, 512 * (tch + 1))

                def mm(dc):
                    def go():
                        if dc == 0:
                            state['ps'] = proj_ps.tile(
                                [128, 512], f32, name="ps_qk", tag="ps_qk")
                        nc.tensor.matmul(
                            state['ps'][:],
                            lhsT=wqk_sb[:, h, dc, :],
                            rhs=xt_sb[:, tch, dc, :],
                            start=(dc == 0),
                            stop=(dc == NDC - 1),
                        )
                    return go

                def cp():
                    nc.vector.tensor_copy(qk1[:, sl], state['ps'][:])
                    # swapped halves, cheap SBUF->SBUF bf16 copies
                    nc.vector.tensor_copy(qk2[0:64, sl], qk1[64:128, sl])
                    nc.vector.tensor_copy(qk2[64:128, sl], qk1[0:64, sl])

                proj_work.extend([mm(dc) for dc in range(NDC)])
                proj_work.append(cp)
                proj_work.append(('marker', ('qk', h, tch)))

            def attn_chunk(h, qk1, qk2, c, o_sb):
                # Attention for head h, one 512-wide t-chunk, causal.
                po = o_ps.tile([DV + 1, 512], f32, name="po", tag="po")
                jmax = 4 * c + 3        # last s-tile index for this chunk
                csl = slice(512 * c, 512 * (c + 1))
                for pair in range(2 * (c + 1)):
                    pS = s_ps.tile([128, 1024], f32, name="pS", tag="pS")
                    pt = ptpool.tile([128, 1024], bf16, name="pt", tag="pt")
                    j0 = 2 * pair
                    # Diagonal s-tiles (relative index r = j - 4c in 0..3) are
                    # fully masked below t-offset 128*r, so S / exp / PV only
                    # cover t in [128*r, 512).  The u=1 tile's output is
                    # COMPACTED to start at psum col 512 so the pair's live
                    # region [f0A : 1024-f0B] stays contiguous and a single
                    # exp op covers it.
                    rA = j0 - 4 * c
                    rB = rA + 1
                    f0A = max(0, 128 * rA)
                    f0B = max(0, 128 * rB)
                    nc.tensor.matmul(
                        pS[:, f0A:512],
                        lhsT=qk2[0:64, 128 * j0:128 * (j0 + 1)],
                        rhs=qk1[0:64, 512 * c + f0A:512 * (c + 1)],
                        start=True,
                        stop=True,
                        tile_position=(0, 0),
                    )
                    nc.tensor.matmul(
                        pS[:, 512:1024 - f0B],
                        lhsT=qk1[64:128, 128 * (j0 + 1):128 * (j0 + 2)],
                        rhs=qk2[64:128, 512 * c + f0B:512 * (c + 1)],
                        start=True,
                        stop=True,
                        tile_position=(64, 0),
                    )
                    nc.scalar.activation(
                        pt[:, f0A:1024 - f0B], pS[:, f0A:1024 - f0B],
                        Exp, scale=DK ** -0.5,
                    )
                    if rA >= 0:
                        # diagonal pair: one multiply with the pre-packed
                        # causal mask (d0 pair at mask cols 0:896, d1 pair at
                        # 896:1280, both laid out to match the compacted pt).
                        m0 = 0 if rA == 0 else 896
                        mw = 896 if rA == 0 else 384
                        nc.vector.tensor_mul(
                            pt[:, f0A:1024 - f0B], pt[:, f0A:1024 - f0B],
                            msk_sb[:, m0:m0 + mw],
                        )
                    for u, j, f0 in ((0, j0, f0A), (1, j0 + 1, f0B)):
                        nc.tensor.matmul(
                            po[:, f0:512],
                            lhsT=v_sb[:, j, h, :],
                            rhs=pt[:, 512 * u + (f0 if u == 0 else 0):
                                   512 * (u + 1) - (0 if u == 0 else f0)],
                            start=(j == 0),
                            stop=(j == jmax),
                        )
                    # fill this pair's exp-wait gap with ~2 projection slots
                    pump(2)
                nc.vector.tensor_copy(o_sb[:, csl], po[:])
                if c == NTC - 1:
                    # one batched 260KB output DMA per head
                    nc.sync.dma_start(ot[h], o_sb[:])

            # Emission order = scheduler priority (see module docstring).
            qks = {h: qk_alloc(h) for h in range(HPC)}
            osbs = {
                h: opool.tile([DV + 1, T], bf16, name=f"o_sb{h}", tag="o_sb")
                for h in range(HPC)
            }
            for tt in range(4):
                enq_v_proj(tt)
            enq_qk_proj_chunk(0, qks[0], 0)
            flush_until(('qk', 0, 0))
            # enqueue the rest in consumption order: remaining heads'
            # chunk-0 projections, then per chunk column c: the v tiles and
            # qk chunks that column consumes.
            for h in range(1, HPC):
                enq_qk_proj_chunk(h, qks[h], 0)
            for c in range(1, NTC):
                for tt in range(4 * c, 4 * (c + 1)):
                    enq_v_proj(tt)
                for h in range(HPC):
                    enq_qk_proj_chunk(h, qks[h], c)
            for c in range(NTC):
                for h in range(HPC):
                    if c > 0 and h == 0:
                        flush_until(('v', 4 * c + 3))
                    flush_until(('qk', h, c))
                    attn_chunk(h, *qks[h], c, osbs[h])
            while proj_work:
                pump(1)

    nc.finalize()
    from concourse.bass_utils import run_bass_kernel_spmd
    _cached = (nc, run_bass_kernel_spmd)
    return _cached


def _prep_core_inputs(x, Wq, Wk, Wv, core):
    b, g = core // 2, core % 2
    xb = x[b].astype(_BF16)                                  # [T, DM]
    xt = np.ascontiguousarray(
        xb.T.reshape(NDC, 128, NTC, 512).transpose(1, 2, 0, 3)  # [p, c, dc, t]
    )
    wq = Wq[HPC * g:HPC * (g + 1)].astype(_BF16)             # [8, DM, 64]
    wk = Wk[HPC * g:HPC * (g + 1)].astype(_BF16)
    wv = Wv[HPC * g:HPC * (g + 1)].astype(_BF16)
    wqk = np.concatenate([wq, wk], axis=2)                   # [h, DM, 128]
    wqk = np.ascontiguousarray(
        wqk.reshape(HPC, NDC, 128, 128).transpose(2, 0, 1, 3)  # [p, h, dc, f]
    )
    wvp = np.ascontiguousarray(
        wv.reshape(HPC, NDC, 128, DV).transpose(2, 1, 0, 3).reshape(128, NDC, 512)
    )
    return {"xt": xt, "wqk": wqk, "wv": wvp, "msk": _mask()}


_mask_cache = None


def _mask():
    # Packed causal masks matching the compacted diagonal-pair layout:
    # cols 0:896  = d0 pair: [r0 tile t 0:512 | r1 tile t 128:512]
    # cols 896:1280 = d1 pair: [r2 tile t 256:512 | r3 tile t 384:512]
    global _mask_cache
    if _mask_cache is None:
        p = np.arange(128)[:, None]
        m = np.zeros((128, 1280), np.float32)
        c = np.arange(512)[None, :]
        m[:, 0:512] = p <= c
        c = np.arange(384)[None, :]
        m[:, 512:896] = p <= c
        c = np.arange(256)[None, :]
        m[:, 896:1152] = p <= c
        c = np.arange(128)[None, :]
        m[:, 1152:1280] = p <= c
        _mask_cache = m.astype(_BF16)
    return _mask_cache


def kernel(x, Wq, Wk, Wv):
    global LAST_EXEC_NS
    nc, run_spmd = _build_program()
    in_maps = [_prep_core_inputs(x, Wq, Wk, Wv, c) for c in range(N_CORES)]
    res = run_spmd(nc, in_maps, list(range(N_CORES)), trace=TRACE)
    global _LAST_RES
    _LAST_RES = res
    LAST_EXEC_NS = res.exec_time_ns

    out = np.empty((B, T, H * DV), np.float32)
    for c in range(N_CORES):
        b, g = c // 2, c % 2
        otc = res.results[c]["ot"].astype(np.float32)  # [8, 65, T]
        o = otc[:, :DV, :] / otc[:, DV:DV + 1, :]      # [h, dv, t]
        out[b, :, 512 * g:512 * (g + 1)] = (
            o.transpose(2, 0, 1).reshape(T, HPC * DV)
        )
    return out
